# revision 24
# baseline (speedup 1.0000x reference)
"""Trainium2 Bass kernel for CrossLayerSharedZOlmoeSparseMoeBlock.

Strategy (expert-parallel, 2 experts/core on 8 cores):
  K1 (device): full routing math, token-sharded 8-way -> comb [T, E] fp32
       - predictor MLP + gumbel argmax in bf16 matmuls
       - router logits via bf16 hi/lo split (3 accumulation chains ->
         ~1e-5 logit error; top-k selection is sensitive to logit error)
       - top-8-of-16 mask via DVE max8 + match_replace, softmax on device
  host: builds per-expert token index lists from device-computed comb
       (the "all-to-all dispatch"), gathers xT columns per expert,
       slices expert weights per core.
  K2 (device): per core, 2 experts: gate/up/down matmuls in bf16 on
       compacted token buffers (padded to 64); gating weight applied
       on-chip at PSUM eviction. Compact outputs returned.
  host: scatter-add compact outputs into y (the "unshard/combine").
"""
import contextlib
import ctypes
import math
import os
import sys
import types

import ml_dtypes
import numpy as np

sys.path.insert(0, "/opt/trn_rl_repo")

# ---------------------------------------------------------------------------
# NTFF profile hook shim (antenv.axon_hooks is absent in this image; bass's
# trace=True path imports it). Lets us read HW exec time via neuron profile.
# ---------------------------------------------------------------------------
_SO_PATH = "/opt/axon/libaxon_pjrt.so"


def _ntff_profile_via_ctypes(so_path):
    try:
        lib = ctypes.CDLL(so_path)
    except OSError:
        return None
    if not hasattr(lib, "axon_start_nrt_profile"):
        return None
    lib.axon_start_nrt_profile.argtypes = [ctypes.POINTER(ctypes.c_int64), ctypes.c_size_t]
    lib.axon_start_nrt_profile.restype = ctypes.c_int64
    lib.axon_stop_nrt_profile.argtypes = [ctypes.c_char_p]
    lib.axon_stop_nrt_profile.restype = ctypes.c_int64

    @contextlib.contextmanager
    def _hook(output_dir, device_ids):
        import jax

        jax.devices()
        if device_ids:
            ids = (ctypes.c_int64 * len(device_ids))(*device_ids)
            rc = lib.axon_start_nrt_profile(ids, len(device_ids))
        else:
            rc = lib.axon_start_nrt_profile(None, 0)
        if rc != 0:
            raise RuntimeError(f"axon_start_nrt_profile rc={rc}")
        try:
            yield
        finally:
            n = lib.axon_stop_nrt_profile(str(output_dir).encode())
            print(f"ntff profile: {n} file(s) -> {output_dir}", file=sys.stderr)

    return _hook


def _install_hook():
    if "antenv.axon_hooks" in sys.modules:
        return
    mod = types.ModuleType("antenv.axon_hooks")
    _h = [_ntff_profile_via_ctypes(_SO_PATH)]
    mod.get_axon_ntff_profile_hook = lambda: _h[0]
    mod.set_axon_ntff_profile_hook = lambda h: _h.__setitem__(0, h)
    sys.modules["antenv.axon_hooks"] = mod
    try:
        import antenv

        antenv.axon_hooks = mod
    except ImportError:
        pass


_install_hook()

import concourse.mybir as mybir  # noqa: E402
import concourse.tile as tile  # noqa: E402
from concourse import bacc  # noqa: E402
from concourse.bass_utils import run_bass_kernel_spmd  # noqa: E402
from concourse.masks import make_identity  # noqa: E402

F32 = mybir.dt.float32
BF16 = mybir.dt.bfloat16
AX = mybir.AxisListType
ALU = mybir.AluOpType
ACTF = mybir.ActivationFunctionType

# problem shapes (hardcoded per contest rules)
B, S, H = 1, 2048, 2048
T = B * S
E, F = 16, 1024
Z, M = 8, 512
TOP_K = 8
EPS = 1e-10
N_CORES = 8
E_LOC = E // N_CORES  # experts per core
TC = T // N_CORES     # tokens per core for routing
P = 128

TRACE = bool(int(os.environ.get("BASSMOE_TRACE", "0")))

_timings = {}

BF = ml_dtypes.bfloat16


def slice_plan(C):
    """Split C into balanced column slices <=512, multiples of 64 (bf16
    matmuls run at 1 cycle/row at any width; balanced widths minimize the
    per-matmul issue floor)."""
    n = math.ceil(C / 512)
    base = C // n // 64 * 64
    widths = [base] * n
    rem = C - base * n
    i = 0
    while rem > 0:
        widths[i % n] += min(64, rem)
        rem -= min(64, rem)
        i += 1
    out, off = [], 0
    for w in widths:
        out.append((off, w))
        off += w
    return out


# ---------------------------------------------------------------------------
# K1: routing kernel (one program, token-sharded across 8 cores)
# ---------------------------------------------------------------------------
def build_k1():
    nc = bacc.Bacc(None, target_bir_lowering=False)
    xt = nc.dram_tensor("xt", [P, H // P, TC], BF16, kind="ExternalInput")
    xlo = nc.dram_tensor("xlo", [P, H // P, TC], BF16, kind="ExternalInput")
    w1t = nc.dram_tensor("w1t", [M // P, P, H // P, P], BF16, kind="ExternalInput")
    w2t = nc.dram_tensor("w2t", [P, M // P, Z], BF16, kind="ExternalInput")
    gwh = nc.dram_tensor("gwh", [P, H // P, E], BF16, kind="ExternalInput")
    gwl = nc.dram_tensor("gwl", [P, H // P, E], BF16, kind="ExternalInput")
    au = nc.dram_tensor("au", [Z, E], BF16, kind="ExternalInput")
    gut = nc.dram_tensor("gut", [Z, TC], F32, kind="ExternalInput")
    b1t = nc.dram_tensor("b1t", [P, M // P], F32, kind="ExternalInput")
    b2t = nc.dram_tensor("b2t", [Z, 1], F32, kind="ExternalInput")
    combo = nc.dram_tensor("combo", [TC // P, P, E], F32, kind="ExternalOutput")

    KH = H // P    # 16
    KM = M // P    # 4
    NCH = TC // P  # token chunks (2)

    with tile.TileContext(nc) as tc:
        with tc.tile_pool(name="const", bufs=1) as const, \
             tc.tile_pool(name="sb", bufs=1) as sb, \
             tc.tile_pool(name="work", bufs=1) as work, \
             tc.tile_pool(name="ps", bufs=2, space="PSUM") as ps, \
             tc.tile_pool(name="psr", bufs=1, space="PSUM") as psr, \
             tc.tile_pool(name="pst", bufs=1, space="PSUM") as pst:
            ident = const.tile([P, P], F32, name="ident")
            make_identity(nc, ident)
            epsc = const.tile([P, 1], F32, name="epsc")
            nc.vector.memset(epsc[:], EPS)

            # PE warmup while input DMAs land
            warm = work.tile([P, 256], BF16, name="warm")
            nc.vector.memset(warm[:], 0.0)
            for _ in range(12):
                wps = ps.tile([P, TC], F32, name="ph")
                nc.tensor.matmul(out=wps[:, :256], lhsT=warm[:, :P], rhs=warm[:],
                                 start=True, stop=True)

            # ---- input DMAs. Few, large transfers; per-queue critical
            # prefix: x halves on sync+gpsimd in parallel, predictor/router
            # weights on scalar. xlo (router chain 2) follows x on gpsimd. ----
            xt_sb = sb.tile([P, KH, TC], BF16, name="xt_sb")
            nc.sync.dma_start(out=xt_sb[:, :6], in_=xt[:, :6])
            nc.gpsimd.dma_start(out=xt_sb[:, 6:11], in_=xt[:, 6:11])
            gut_sb = sb.tile([Z, TC], F32, name="gut_sb")
            nc.scalar.dma_start(out=gut_sb[:], in_=gut[:])
            gwh_sb = sb.tile([P, KH, E], BF16, name="gwh_sb")
            nc.scalar.dma_start(out=gwh_sb[:], in_=gwh[:])
            nc.scalar.dma_start(out=xt_sb[:, 11:], in_=xt[:, 11:])
            gwl_sb = sb.tile([P, KH, E], BF16, name="gwl_sb")
            nc.scalar.dma_start(out=gwl_sb[:], in_=gwl[:])
            w1t_sb = sb.tile([P, M // P, KH, P], BF16, name="w1t_sb")
            nc.scalar.dma_start(out=w1t_sb[:, 0], in_=w1t[0])
            nc.scalar.dma_start(out=w1t_sb[:, 1:], in_=w1t[1:])
            xlo_sb = sb.tile([P, KH, TC], BF16, name="xlo_sb")
            nc.gpsimd.dma_start(out=xlo_sb[:], in_=xlo[:])
            w2t_sb = sb.tile([P, KM, Z], BF16, name="w2t_sb")
            nc.sync.dma_start(out=w2t_sb[:], in_=w2t[:])
            b1t_sb = sb.tile([P, M // P], F32, name="b1t_sb")
            nc.sync.dma_start(out=b1t_sb[:], in_=b1t[:])
            b2t_sb = sb.tile([Z, 1], F32, name="b2t_sb")
            nc.sync.dma_start(out=b2t_sb[:], in_=b2t[:])
            au_sb = sb.tile([Z, E], BF16, name="au_sb")
            nc.sync.dma_start(out=au_sb[:], in_=au[:])

            # gumbel first (only needs gut; groups both Ln table ops before
            # the predictor Silus to avoid ACT table thrash)
            gv = work.tile([Z, TC], F32, name="gv")
            nc.scalar.activation(out=gv[:], in_=gut_sb[:], func=ACTF.Ln,
                                 bias=epsc[:Z, 0:1], scale=1.0)
            gw = work.tile([Z, TC], F32, name="gw")
            nc.scalar.activation(out=gw[:], in_=gv[:], func=ACTF.Ln,
                                 bias=epsc[:Z, 0:1], scale=-1.0)

            # router main term: rlT [E, TC] = gw_hi.T@x_hi + gw_hi.T@x_lo
            #                               + gw_lo.T@x_hi  (bf16 hi/lo split)
            prl = psr.tile([E, TC], F32, name="prl")
            for k in range(KH):
                nc.tensor.matmul(out=prl[:], lhsT=gwh_sb[:, k, :],
                                 rhs=xt_sb[:, k, :], start=(k == 0), stop=False)
            for k in range(KH):
                nc.tensor.matmul(out=prl[:], lhsT=gwh_sb[:, k, :],
                                 rhs=xlo_sb[:, k, :], start=False, stop=False)
            for k in range(KH):
                nc.tensor.matmul(out=prl[:], lhsT=gwl_sb[:, k, :],
                                 rhs=xt_sb[:, k, :], start=False, stop=False)

            # predictor: h1T = silu(W1 @ xT + b1)  [M, TC]
            h1t = sb.tile([P, KM, TC], BF16, name="h1t")
            for m in range(KM):
                ph = ps.tile([P, TC], F32, name="ph")
                for k in range(KH):
                    nc.tensor.matmul(
                        out=ph[:],
                        lhsT=w1t_sb[:, m, k, :],
                        rhs=xt_sb[:, k, :],
                        start=(k == 0), stop=(k == KH - 1),
                    )
                nc.scalar.activation(
                    out=h1t[:, m, :], in_=ph[:], func=ACTF.Silu,
                    bias=b1t_sb[:, m:m + 1], scale=1.0,
                )

            # zT = W2 @ h1T + b2   [Z, TC]
            pz = ps.tile([Z, TC], F32, name="pz")
            for mk in range(KM):
                nc.tensor.matmul(
                    out=pz[:], lhsT=w2t_sb[:, mk, :], rhs=h1t[:, mk, :],
                    start=(mk == 0), stop=(mk == KM - 1),
                )
            zt = work.tile([Z, TC], F32, name="zt")
            nc.scalar.activation(out=zt[:], in_=pz[:], func=ACTF.Identity,
                                 bias=b2t_sb[:, 0:1], scale=1.0)

            # sT = zT - w  (= z + gumbel)
            st = work.tile([Z, TC], F32, name="st")
            nc.vector.tensor_tensor(out=st[:], in0=zt[:], in1=gw[:], op=ALU.subtract)

            # transpose sT -> s [tok, Z] per 128-token chunk
            s_sb = work.tile([P, NCH, Z], F32, name="s_sb")
            for c in range(NCH):
                pt = pst.tile([P, Z], F32, name="pt")
                nc.tensor.transpose(
                    out=pt[:], in_=st[:, c * P:(c + 1) * P], identity=ident[:Z, :Z])
                nc.vector.tensor_copy(out=s_sb[:, c, :], in_=pt[:])

            # onehot of argmax over Z (per token)
            rmax = work.tile([P, NCH], F32, name="rmax")
            nc.vector.tensor_reduce(out=rmax[:], in_=s_sb[:], axis=AX.X, op=ALU.max)
            onehot = work.tile([P, NCH, Z], F32, name="onehot")
            for c in range(NCH):
                nc.vector.tensor_scalar(
                    out=onehot[:, c, :], in0=s_sb[:, c, :],
                    scalar1=rmax[:, c:c + 1], scalar2=None, op0=ALU.is_equal)

            # onehotT [Z, chunk*P] (bf16) for router-bias matmul
            ohT = work.tile([Z, NCH * P], BF16, name="ohT")
            for c in range(NCH):
                po = pst.tile([Z, P], F32, name="po")
                nc.tensor.transpose(
                    out=po[:], in_=onehot[:, c, :], identity=ident[:P, :P])
                nc.vector.tensor_copy(out=ohT[:, c * P:(c + 1) * P], in_=po[:])

            # rlT += (alpha U).T @ onehotT  -> finish accumulation
            nc.tensor.matmul(out=prl[:], lhsT=au_sb[:], rhs=ohT[:],
                             start=False, stop=True)
            rlt = work.tile([E, TC], F32, name="rlt")
            nc.vector.tensor_copy(out=rlt[:], in_=prl[:])

            # transpose rlT -> rl [tok, E] per chunk
            rl_all = work.tile([P, NCH, E], F32, name="rl_all")
            for c in range(NCH):
                pr = pst.tile([P, E], F32, name="pr")
                nc.tensor.transpose(
                    out=pr[:], in_=rlt[:, c * P:(c + 1) * P], identity=ident[:E, :E])
                nc.vector.tensor_copy(out=rl_all[:, c, :], in_=pr[:])

            def bcast(t):
                return t[:, :, 0:1].to_broadcast([P, NCH, E])

            # top-8 selection via DVE max8 + match_replace
            rep = work.tile([P, NCH, E], F32, name="rep")
            for c in range(NCH):
                mx8 = work.tile([P, 8], F32, name="mx8")
                nc.vector.max(out=mx8[:], in_=rl_all[:, c, :])
                nc.vector.match_replace(out=rep[:, c, :], in_to_replace=mx8[:],
                                        in_values=rl_all[:, c, :], imm_value=-1e30)

            # softmax over E
            mxn = work.tile([P, NCH, 1], F32, name="mxn")
            nc.vector.tensor_reduce(out=mxn[:, :, 0], in_=rl_all[:], axis=AX.X,
                                    op=ALU.max, negate=True)
            ex = work.tile([P, NCH, E], F32, name="ex")
            for c in range(NCH):
                nc.scalar.activation(out=ex[:, c, :], in_=rl_all[:, c, :],
                                     func=ACTF.Exp, bias=mxn[:, c, 0:1], scale=1.0)
            sm = work.tile([P, NCH, 1], F32, name="sm")
            nc.vector.tensor_reduce(out=sm[:, :, 0], in_=ex[:], axis=AX.X,
                                    op=ALU.add)
            inv = work.tile([P, NCH, 1], F32, name="inv")
            nc.vector.reciprocal(out=inv[:], in_=sm[:])

            cmb = work.tile([P, NCH, E], F32, name="cmb")
            nc.vector.tensor_tensor(out=cmb[:], in0=rl_all[:], in1=rep[:],
                                    op=ALU.not_equal)
            nc.vector.tensor_tensor(out=cmb[:], in0=cmb[:], in1=ex[:],
                                    op=ALU.mult)
            nc.vector.tensor_tensor(out=cmb[:], in0=cmb[:], in1=bcast(inv),
                                    op=ALU.mult)
            for c in range(NCH):
                nc.sync.dma_start(out=combo[c], in_=cmb[:, c, :])
    nc.compile()
    return nc


# ---------------------------------------------------------------------------
# K2: expert kernel (expert-parallel; C tokens per expert, compile-time C)
# ---------------------------------------------------------------------------
def build_k2(C):
    CR = P * math.ceil(C / P)  # down-proj processes 128-token chunks
    CC = CR // P
    CS = slice_plan(C)    # gate/up column slices
    KH = H // P           # 16
    KF = F // P           # 8
    MF = F // P           # 8 m-chunks for gate/up
    HS = H // 512         # 4

    nc = bacc.Bacc(None, target_bir_lowering=False)
    # xgt per expert, slice-blocked: [s0 k-half1 | s0 k-half2 | s1 | s2 ...]
    xgt = nc.dram_tensor("xgt", [E_LOC, P * KH * C], BF16, kind="ExternalInput")
    # gate+up weights packed per m-chunk: [e, m, p, {g,u}, k, f]
    wgut = nc.dram_tensor("wgut", [E_LOC, MF, P, 2, KH, P], BF16,
                          kind="ExternalInput")
    # down weights: [e, p, hs-pair, {hs}, k, f512]
    wdt = nc.dram_tensor("wdt", [E_LOC, P, 2, 2, KF, 512], BF16,
                         kind="ExternalInput")
    wv = nc.dram_tensor("wv", [E_LOC, P, CC], F32, kind="ExternalInput")
    outc = nc.dram_tensor("outc", [E_LOC, CC, HS, P, 512], BF16, kind="ExternalOutput")

    with tile.TileContext(nc) as tc:
        with tc.tile_pool(name="xg", bufs=2) as xg_pool, \
             tc.tile_pool(name="act", bufs=2) as act_pool, \
             tc.tile_pool(name="wgu", bufs=2) as wgu_pool, \
             tc.tile_pool(name="wd", bufs=2) as wd_pool, \
             tc.tile_pool(name="wvp", bufs=2) as wv_pool, \
             tc.tile_pool(name="tmp", bufs=3) as tmp_pool, \
             tc.tile_pool(name="ev", bufs=4) as ev_pool, \
             tc.tile_pool(name="psg", bufs=2, space="PSUM") as psg, \
             tc.tile_pool(name="psu", bufs=2, space="PSUM") as psu, \
             tc.tile_pool(name="psd", bufs=3, space="PSUM") as psd:
            # PE warmup: keep HAM at 8/8 while initial DMAs land
            warm = tmp_pool.tile([P, 384], BF16, name="warm")
            nc.vector.memset(warm[:], 0.0)
            for _ in range(16):
                wps = psd.tile([P, 512], F32, name="pd")
                nc.tensor.matmul(out=wps[:, :384], lhsT=warm[:, :P],
                                 rhs=warm[:, :384], start=True, stop=True)

            # both experts' gathers + gating weights issued up front: slice-0
            # k-halves first (minimal critical prefix), remaining slices
            # alternate queues; e1's transfers queue naturally behind e0's.
            xgt_sbs, wv_sbs = [], []
            for e in range(E_LOC):
                xgt_sb = xg_pool.tile([P, KH, C], BF16, name="xgt_sb")
                off = 0
                for si, (c0, cw) in enumerate(CS):
                    if si == 0:
                        n1 = P * (KH // 2) * cw
                        nc.sync.dma_start(out=xgt_sb[:, :KH // 2, c0:c0 + cw],
                                          in_=xgt[e, off:off + n1])
                        nc.gpsimd.dma_start(out=xgt_sb[:, KH // 2:, c0:c0 + cw],
                                            in_=xgt[e, off + n1:off + 2 * n1])
                        off += 2 * n1
                    else:
                        n = P * KH * cw
                        q = nc.sync if si % 2 == 1 else nc.gpsimd
                        q.dma_start(out=xgt_sb[:, :, c0:c0 + cw],
                                    in_=xgt[e, off:off + n])
                        off += n
                wv_sb = wv_pool.tile([P, CC], F32, name="wv_sb")
                nc.gpsimd.dma_start(out=wv_sb[:], in_=wv[e])
                xgt_sbs.append(xgt_sb)
                wv_sbs.append(wv_sb)

            for e in range(E_LOC):
                xgt_sb = xgt_sbs[e]
                wv_sb = wv_sbs[e]
                actT = act_pool.tile([P, KF, CR], BF16, name="actT")
                if CR > C:
                    # zero the padded token tail so the down matmul reads 0s
                    nc.vector.memset(actT[:, :, C:], 0.0)

                # gate/up projections + silu*up, output actT [F, C]
                for m in range(MF):
                    wgu_sb = wgu_pool.tile([P, 2, KH, P], BF16, name="wgu_sb")
                    nc.scalar.dma_start(out=wgu_sb[:], in_=wgut[e, m])
                    for (c0, cw) in CS:
                        pg = psg.tile([P, 512], F32, name="pg")[:, :cw]
                        pu = psu.tile([P, 512], F32, name="pu")[:, :cw]
                        for k in range(KH):
                            nc.tensor.matmul(
                                out=pg[:], lhsT=wgu_sb[:, 0, k, :],
                                rhs=xgt_sb[:, k, c0:c0 + cw],
                                start=(k == 0), stop=(k == KH - 1))
                        for k in range(KH):
                            nc.tensor.matmul(
                                out=pu[:], lhsT=wgu_sb[:, 1, k, :],
                                rhs=xgt_sb[:, k, c0:c0 + cw],
                                start=(k == 0), stop=(k == KH - 1))
                        sg = tmp_pool.tile([P, 512], F32, name="sg")[:, :cw]
                        nc.scalar.activation(out=sg[:], in_=pg[:], func=ACTF.Silu,
                                             bias=0.0, scale=1.0)
                        nc.vector.tensor_tensor(
                            out=actT[:, m, c0:c0 + cw], in0=sg[:], in1=pu[:],
                            op=ALU.mult)

                # down projection, gating scale at eviction, compact out rows
                for hc in range(2):
                    wd_sb = wd_pool.tile([P, 2, KF, 512], BF16, name="wd_sb")
                    nc.gpsimd.dma_start(out=wd_sb[:], in_=wdt[e, :, hc])
                    for hi in range(2):
                        hs = hc * 2 + hi
                        for cc in range(CC):
                            pd = psd.tile([P, 512], F32, name="pd")
                            for k in range(KF):
                                nc.tensor.matmul(
                                    out=pd[:], lhsT=actT[:, k, cc * P:(cc + 1) * P],
                                    rhs=wd_sb[:, hi, k, :],
                                    start=(k == 0), stop=(k == KF - 1))
                            ev = ev_pool.tile([P, 512], BF16, name="ev")
                            nc.scalar.activation(out=ev[:], in_=pd[:], func=ACTF.Copy,
                                                 bias=0.0, scale=wv_sb[:, cc:cc + 1])
                            evq = nc.sync if (cc % 2 == 0) else nc.scalar
                            evq.dma_start(out=outc[e, cc, hs], in_=ev[:])
    nc.compile()
    return nc


# ---------------------------------------------------------------------------
# host orchestration
# ---------------------------------------------------------------------------
def _il(x, p=P):
    """[R, N] -> [p, R//p, N] with row r = k*p + part."""
    r, n = x.shape
    return np.ascontiguousarray(x.reshape(r // p, p, n).transpose(1, 0, 2))


_k2_cache = {}


def kernel(hidden_states, gumbel_u, W1, b1, W2, b2, gate_w, U, alpha, Wg, Wu, Wd):
    import time as _time

    t_start = _time.time()
    x = np.asarray(hidden_states, np.float32).reshape(T, H)

    # ---- host prep for K1 ----
    # xT interleaved: [128, H/128, T] with (p, k, t) = x[t, k*128+p]
    xT_il = np.ascontiguousarray(np.asarray(x).reshape(T, H // P, P).transpose(2, 1, 0))
    xT_hi = xT_il.astype(BF)
    xT_lo = (xT_il - xT_hi.astype(np.float32)).astype(BF)
    w1t = np.asarray(W1, np.float32).reshape(M // P, P, H // P, P).transpose(0, 3, 2, 1)
    w1t = np.ascontiguousarray(w1t).astype(BF)
    w2t = _il(np.ascontiguousarray(np.asarray(W2, np.float32).T)).astype(BF)
    gwt = _il(np.ascontiguousarray(np.asarray(gate_w, np.float32).T))    # [128, 16, 16]
    gwh = gwt.astype(BF)
    gwl = (gwt - gwh.astype(np.float32)).astype(BF)
    au = (np.float32(alpha) * np.asarray(U, np.float32)).astype(BF)
    guT = np.ascontiguousarray(np.asarray(gumbel_u, np.float32).T)       # [8, T]
    b1t = np.ascontiguousarray(np.asarray(b1, np.float32).reshape(M // P, P).T)
    b2t = np.ascontiguousarray(np.asarray(b2, np.float32).reshape(Z, 1))

    in_maps1 = []
    for c in range(N_CORES):
        sl = slice(c * TC, (c + 1) * TC)
        in_maps1.append({
            "xt": np.ascontiguousarray(xT_hi[:, :, sl]),
            "xlo": np.ascontiguousarray(xT_lo[:, :, sl]),
            "w1t": w1t, "w2t": w2t, "gwh": gwh, "gwl": gwl, "au": au,
            "gut": np.ascontiguousarray(guT[:, sl]),
            "b1t": b1t, "b2t": b2t,
        })

    t0 = _time.time()
    nc1 = _k2_cache.get("k1")
    if nc1 is None:
        nc1 = build_k1()
        _k2_cache["k1"] = nc1
    _timings["k1_build"] = _time.time() - t0

    t0 = _time.time()
    res1 = run_bass_kernel_spmd(nc1, in_maps1, list(range(N_CORES)), trace=TRACE)
    _timings["k1_run"] = _time.time() - t0
    if TRACE:
        _timings["k1_hw_ns"] = res1.exec_time_ns

    comb = np.concatenate(
        [res1.results[c]["combo"].reshape(TC, E) for c in range(N_CORES)], axis=0)

    # ---- host routing: index lists + dispatch ----
    t0 = _time.time()
    idxs, wvals, counts = [], [], []
    for e in range(E):
        ie = np.nonzero(comb[:, e] > 0)[0].astype(np.int64)
        idxs.append(ie)
        wvals.append(comb[ie, e].astype(np.float32))
        counts.append(len(ie))
    C = max(128, 64 * math.ceil(max(counts) / 64))
    CC = math.ceil(C / P)
    CR = CC * P

    idx_pad = np.zeros((E, C), np.int64)
    w_pad = np.zeros((E, CR), np.float32)
    for e in range(E):
        n = counts[e]
        idx_pad[e, :n] = idxs[e]
        w_pad[e, :n] = wvals[e]

    # weights, transposed+interleaved+blocked (built once; per-core slices are views)
    MF, HS = F // P, H // 512
    # [E, MF, 128(p), 16(k), 128(f)]: wgt[e,m,p,k,j] = Wg[e, m*128+j, k*128+p]
    WgT = np.asarray(Wg, np.float32).reshape(E, MF, P, H // P, P).transpose(0, 1, 4, 3, 2)
    WuT = np.asarray(Wu, np.float32).reshape(E, MF, P, H // P, P).transpose(0, 1, 4, 3, 2)
    # gate+up packed per m: [E, MF, P, {g,u}, KH, P]
    Wgu = np.stack([WgT.astype(BF), WuT.astype(BF)], axis=3)
    # [E, HS, 128(p), 8(k), 512(h)]: wdt[e,s,p,k,j] = Wd[e, s*512+j, k*128+p]
    WdT = np.asarray(Wd, np.float32).reshape(E, HS, 512, F // P, P).transpose(0, 1, 4, 3, 2)
    # -> [E, P, 2(hc), 2(hi), KF, 512] for single-dma-per-hs-pair loading
    WdT2 = np.ascontiguousarray(WdT.astype(BF).transpose(0, 2, 1, 3, 4)).reshape(
        E, P, 2, 2, F // P, 512)

    in_maps2 = []
    for c in range(N_CORES):
        es = [E_LOC * c + i for i in range(E_LOC)]
        plan = slice_plan(C)
        xg_list = []
        for e in es:
            g = xT_hi[:, :, idx_pad[e]]                               # [128,16,C]
            blocks = []
            for si, (c0, cw) in enumerate(plan):
                blk = g[:, :, c0:c0 + cw]
                if si == 0:
                    blocks.append(np.ascontiguousarray(blk[:, :8, :]).reshape(-1))
                    blocks.append(np.ascontiguousarray(blk[:, 8:, :]).reshape(-1))
                else:
                    blocks.append(np.ascontiguousarray(blk).reshape(-1))
            xg_list.append(np.concatenate(blocks))
        xg = np.stack(xg_list)                                        # [2, P*KH*C]
        wvc = np.stack([np.ascontiguousarray(w_pad[e].reshape(CC, P).T)
                        for e in es])                                  # [2,128,CC]
        in_maps2.append({
            "xgt": xg,
            "wgut": Wgu[es[0]:es[-1] + 1],
            "wdt": WdT2[es[0]:es[-1] + 1],
            "wv": wvc,
        })
    _timings["dispatch"] = _time.time() - t0

    t0 = _time.time()
    nc2 = _k2_cache.get(("k2", C))
    if nc2 is None:
        nc2 = build_k2(C)
        _k2_cache[("k2", C)] = nc2
    _timings["k2_build"] = _time.time() - t0

    t0 = _time.time()
    res2 = run_bass_kernel_spmd(nc2, in_maps2, list(range(N_CORES)), trace=TRACE)
    _timings["k2_run"] = _time.time() - t0
    if TRACE:
        _timings["k2_hw_ns"] = res2.exec_time_ns

    # ---- host combine (unshard) ----
    t0 = _time.time()
    y = np.zeros((T, H), np.float32)
    for e in range(E):
        c, i = divmod(e, E_LOC)
        oc = res2.results[c]["outc"][i]          # [CC, HS, 128, 512] bf16
        oc = oc.transpose(0, 2, 1, 3).reshape(-1, H).astype(np.float32)
        n = counts[e]
        y[idxs[e]] += oc[:n]
    _timings["combine"] = _time.time() - t0
    _timings["total"] = _time.time() - t_start
    return y.reshape(B, S, H)


# revision 25
# speedup vs baseline: 1.0236x; 1.0236x over previous
"""Trainium2 Bass kernel for CrossLayerSharedZOlmoeSparseMoeBlock.

Strategy (expert-parallel, 2 experts/core on 8 cores):
  K1 (device): full routing math, token-sharded 8-way -> comb [T, E] fp32
       - predictor MLP + gumbel argmax in bf16 matmuls
       - router logits via bf16 hi/lo split (3 accumulation chains ->
         ~1e-5 logit error; top-k selection is sensitive to logit error)
       - top-8-of-16 mask via DVE max8 + match_replace, softmax on device
  host: builds per-expert token index lists from device-computed comb
       (the "all-to-all dispatch"), gathers xT columns per expert,
       slices expert weights per core.
  K2 (device): per core, 2 experts: gate/up/down matmuls in bf16 on
       compacted token buffers (padded to 64); gating weight applied
       on-chip at PSUM eviction. Compact outputs returned.
  host: scatter-add compact outputs into y (the "unshard/combine").
"""
import contextlib
import ctypes
import math
import os
import sys
import types

import ml_dtypes
import numpy as np

sys.path.insert(0, "/opt/trn_rl_repo")

# ---------------------------------------------------------------------------
# NTFF profile hook shim (antenv.axon_hooks is absent in this image; bass's
# trace=True path imports it). Lets us read HW exec time via neuron profile.
# ---------------------------------------------------------------------------
_SO_PATH = "/opt/axon/libaxon_pjrt.so"


def _ntff_profile_via_ctypes(so_path):
    try:
        lib = ctypes.CDLL(so_path)
    except OSError:
        return None
    if not hasattr(lib, "axon_start_nrt_profile"):
        return None
    lib.axon_start_nrt_profile.argtypes = [ctypes.POINTER(ctypes.c_int64), ctypes.c_size_t]
    lib.axon_start_nrt_profile.restype = ctypes.c_int64
    lib.axon_stop_nrt_profile.argtypes = [ctypes.c_char_p]
    lib.axon_stop_nrt_profile.restype = ctypes.c_int64

    @contextlib.contextmanager
    def _hook(output_dir, device_ids):
        import jax

        jax.devices()
        if device_ids:
            ids = (ctypes.c_int64 * len(device_ids))(*device_ids)
            rc = lib.axon_start_nrt_profile(ids, len(device_ids))
        else:
            rc = lib.axon_start_nrt_profile(None, 0)
        if rc != 0:
            raise RuntimeError(f"axon_start_nrt_profile rc={rc}")
        try:
            yield
        finally:
            n = lib.axon_stop_nrt_profile(str(output_dir).encode())
            print(f"ntff profile: {n} file(s) -> {output_dir}", file=sys.stderr)

    return _hook


def _install_hook():
    if "antenv.axon_hooks" in sys.modules:
        return
    mod = types.ModuleType("antenv.axon_hooks")
    _h = [_ntff_profile_via_ctypes(_SO_PATH)]
    mod.get_axon_ntff_profile_hook = lambda: _h[0]
    mod.set_axon_ntff_profile_hook = lambda h: _h.__setitem__(0, h)
    sys.modules["antenv.axon_hooks"] = mod
    try:
        import antenv

        antenv.axon_hooks = mod
    except ImportError:
        pass


_install_hook()

import concourse.mybir as mybir  # noqa: E402
import concourse.tile as tile  # noqa: E402
from concourse import bacc  # noqa: E402
from concourse.bass_utils import run_bass_kernel_spmd  # noqa: E402
from concourse.masks import make_identity  # noqa: E402

F32 = mybir.dt.float32
BF16 = mybir.dt.bfloat16
AX = mybir.AxisListType
ALU = mybir.AluOpType
ACTF = mybir.ActivationFunctionType

# problem shapes (hardcoded per contest rules)
B, S, H = 1, 2048, 2048
T = B * S
E, F = 16, 1024
Z, M = 8, 512
TOP_K = 8
EPS = 1e-10
N_CORES = 8
E_LOC = E // N_CORES  # experts per core
TC = T // N_CORES     # tokens per core for routing
P = 128

TRACE = bool(int(os.environ.get("BASSMOE_TRACE", "0")))

_timings = {}

BF = ml_dtypes.bfloat16


def slice_plan(C):
    """Split C into balanced column slices <=512, multiples of 64 (bf16
    matmuls run at 1 cycle/row at any width; balanced widths minimize the
    per-matmul issue floor)."""
    n = math.ceil(C / 512)
    base = C // n // 64 * 64
    widths = [base] * n
    rem = C - base * n
    i = 0
    while rem > 0:
        widths[i % n] += min(64, rem)
        rem -= min(64, rem)
        i += 1
    out, off = [], 0
    for w in widths:
        out.append((off, w))
        off += w
    return out


# ---------------------------------------------------------------------------
# K1: routing kernel (one program, token-sharded across 8 cores)
# ---------------------------------------------------------------------------
def build_k1():
    nc = bacc.Bacc(None, target_bir_lowering=False)
    xt = nc.dram_tensor("xt", [P, H // P, TC], BF16, kind="ExternalInput")
    xlo = nc.dram_tensor("xlo", [P, H // P, TC], BF16, kind="ExternalInput")
    w1t = nc.dram_tensor("w1t", [M // P, P, H // P, P], BF16, kind="ExternalInput")
    w2t = nc.dram_tensor("w2t", [P, M // P, Z], BF16, kind="ExternalInput")
    gwh = nc.dram_tensor("gwh", [P, H // P, E], BF16, kind="ExternalInput")
    gwl = nc.dram_tensor("gwl", [P, H // P, E], BF16, kind="ExternalInput")
    au = nc.dram_tensor("au", [Z, E], BF16, kind="ExternalInput")
    gut = nc.dram_tensor("gut", [Z, TC], F32, kind="ExternalInput")
    b1t = nc.dram_tensor("b1t", [P, M // P], F32, kind="ExternalInput")
    b2t = nc.dram_tensor("b2t", [Z, 1], F32, kind="ExternalInput")
    combo = nc.dram_tensor("combo", [TC // P, P, E], F32, kind="ExternalOutput")

    KH = H // P    # 16
    KM = M // P    # 4
    NCH = TC // P  # token chunks (2)

    with tile.TileContext(nc) as tc:
        with tc.tile_pool(name="const", bufs=1) as const, \
             tc.tile_pool(name="sb", bufs=1) as sb, \
             tc.tile_pool(name="work", bufs=1) as work, \
             tc.tile_pool(name="ps", bufs=2, space="PSUM") as ps, \
             tc.tile_pool(name="psr", bufs=1, space="PSUM") as psr, \
             tc.tile_pool(name="pst", bufs=1, space="PSUM") as pst:
            ident = const.tile([P, P], F32, name="ident")
            make_identity(nc, ident)
            epsc = const.tile([P, 1], F32, name="epsc")
            nc.vector.memset(epsc[:], EPS)

            # PE warmup while input DMAs land
            warm = work.tile([P, 256], BF16, name="warm")
            nc.vector.memset(warm[:], 0.0)
            for _ in range(12):
                wps = ps.tile([P, TC], F32, name="ph")
                nc.tensor.matmul(out=wps[:, :256], lhsT=warm[:, :P], rhs=warm[:],
                                 start=True, stop=True)

            # ---- input DMAs. Few, large transfers; per-queue critical
            # prefix: x halves on sync+gpsimd in parallel, predictor/router
            # weights on scalar. xlo (router chain 2) follows x on gpsimd. ----
            xt_sb = sb.tile([P, KH, TC], BF16, name="xt_sb")
            nc.sync.dma_start(out=xt_sb[:, :KH // 2], in_=xt[:, :KH // 2])
            nc.gpsimd.dma_start(out=xt_sb[:, KH // 2:], in_=xt[:, KH // 2:])
            gut_sb = sb.tile([Z, TC], F32, name="gut_sb")
            nc.scalar.dma_start(out=gut_sb[:], in_=gut[:])
            gwh_sb = sb.tile([P, KH, E], BF16, name="gwh_sb")
            nc.scalar.dma_start(out=gwh_sb[:], in_=gwh[:])
            gwl_sb = sb.tile([P, KH, E], BF16, name="gwl_sb")
            nc.scalar.dma_start(out=gwl_sb[:], in_=gwl[:])
            w1t_sb = sb.tile([P, M // P, KH, P], BF16, name="w1t_sb")
            nc.scalar.dma_start(out=w1t_sb[:, 0], in_=w1t[0])
            nc.scalar.dma_start(out=w1t_sb[:, 1:], in_=w1t[1:])
            xlo_sb = sb.tile([P, KH, TC], BF16, name="xlo_sb")
            nc.gpsimd.dma_start(out=xlo_sb[:], in_=xlo[:])
            w2t_sb = sb.tile([P, KM, Z], BF16, name="w2t_sb")
            nc.sync.dma_start(out=w2t_sb[:], in_=w2t[:])
            b1t_sb = sb.tile([P, M // P], F32, name="b1t_sb")
            nc.sync.dma_start(out=b1t_sb[:], in_=b1t[:])
            b2t_sb = sb.tile([Z, 1], F32, name="b2t_sb")
            nc.sync.dma_start(out=b2t_sb[:], in_=b2t[:])
            au_sb = sb.tile([Z, E], BF16, name="au_sb")
            nc.sync.dma_start(out=au_sb[:], in_=au[:])

            # gumbel first (only needs gut; groups both Ln table ops before
            # the predictor Silus to avoid ACT table thrash)
            gv = work.tile([Z, TC], F32, name="gv")
            nc.scalar.activation(out=gv[:], in_=gut_sb[:], func=ACTF.Ln,
                                 bias=epsc[:Z, 0:1], scale=1.0)
            gw = work.tile([Z, TC], F32, name="gw")
            nc.scalar.activation(out=gw[:], in_=gv[:], func=ACTF.Ln,
                                 bias=epsc[:Z, 0:1], scale=-1.0)

            # router main term: rlT [E, TC] = gw_hi.T@x_hi + gw_hi.T@x_lo
            #                               + gw_lo.T@x_hi  (bf16 hi/lo split)
            prl = psr.tile([E, TC], F32, name="prl")
            for k in range(KH):
                nc.tensor.matmul(out=prl[:], lhsT=gwh_sb[:, k, :],
                                 rhs=xt_sb[:, k, :], start=(k == 0), stop=False)
            for k in range(KH):
                nc.tensor.matmul(out=prl[:], lhsT=gwh_sb[:, k, :],
                                 rhs=xlo_sb[:, k, :], start=False, stop=False)
            for k in range(KH):
                nc.tensor.matmul(out=prl[:], lhsT=gwl_sb[:, k, :],
                                 rhs=xt_sb[:, k, :], start=False, stop=False)

            # predictor: h1T = silu(W1 @ xT + b1)  [M, TC]
            h1t = sb.tile([P, KM, TC], BF16, name="h1t")
            for m in range(KM):
                ph = ps.tile([P, TC], F32, name="ph")
                for k in range(KH):
                    nc.tensor.matmul(
                        out=ph[:],
                        lhsT=w1t_sb[:, m, k, :],
                        rhs=xt_sb[:, k, :],
                        start=(k == 0), stop=(k == KH - 1),
                    )
                nc.scalar.activation(
                    out=h1t[:, m, :], in_=ph[:], func=ACTF.Silu,
                    bias=b1t_sb[:, m:m + 1], scale=1.0,
                )

            # zT = W2 @ h1T + b2   [Z, TC]
            pz = ps.tile([Z, TC], F32, name="pz")
            for mk in range(KM):
                nc.tensor.matmul(
                    out=pz[:], lhsT=w2t_sb[:, mk, :], rhs=h1t[:, mk, :],
                    start=(mk == 0), stop=(mk == KM - 1),
                )
            zt = work.tile([Z, TC], F32, name="zt")
            nc.scalar.activation(out=zt[:], in_=pz[:], func=ACTF.Identity,
                                 bias=b2t_sb[:, 0:1], scale=1.0)

            # sT = zT - w  (= z + gumbel)
            st = work.tile([Z, TC], F32, name="st")
            nc.vector.tensor_tensor(out=st[:], in0=zt[:], in1=gw[:], op=ALU.subtract)

            # transpose sT -> s [tok, Z] per 128-token chunk
            s_sb = work.tile([P, NCH, Z], F32, name="s_sb")
            for c in range(NCH):
                pt = pst.tile([P, Z], F32, name="pt")
                nc.tensor.transpose(
                    out=pt[:], in_=st[:, c * P:(c + 1) * P], identity=ident[:Z, :Z])
                nc.vector.tensor_copy(out=s_sb[:, c, :], in_=pt[:])

            # onehot of argmax over Z (per token)
            rmax = work.tile([P, NCH], F32, name="rmax")
            nc.vector.tensor_reduce(out=rmax[:], in_=s_sb[:], axis=AX.X, op=ALU.max)
            onehot = work.tile([P, NCH, Z], F32, name="onehot")
            for c in range(NCH):
                nc.vector.tensor_scalar(
                    out=onehot[:, c, :], in0=s_sb[:, c, :],
                    scalar1=rmax[:, c:c + 1], scalar2=None, op0=ALU.is_equal)

            # onehotT [Z, chunk*P] (bf16) for router-bias matmul
            ohT = work.tile([Z, NCH * P], BF16, name="ohT")
            for c in range(NCH):
                po = pst.tile([Z, P], F32, name="po")
                nc.tensor.transpose(
                    out=po[:], in_=onehot[:, c, :], identity=ident[:P, :P])
                nc.vector.tensor_copy(out=ohT[:, c * P:(c + 1) * P], in_=po[:])

            # rlT += (alpha U).T @ onehotT  -> finish accumulation
            nc.tensor.matmul(out=prl[:], lhsT=au_sb[:], rhs=ohT[:],
                             start=False, stop=True)
            rlt = work.tile([E, TC], F32, name="rlt")
            nc.vector.tensor_copy(out=rlt[:], in_=prl[:])

            # transpose rlT -> rl [tok, E] per chunk
            rl_all = work.tile([P, NCH, E], F32, name="rl_all")
            for c in range(NCH):
                pr = pst.tile([P, E], F32, name="pr")
                nc.tensor.transpose(
                    out=pr[:], in_=rlt[:, c * P:(c + 1) * P], identity=ident[:E, :E])
                nc.vector.tensor_copy(out=rl_all[:, c, :], in_=pr[:])

            def bcast(t):
                return t[:, :, 0:1].to_broadcast([P, NCH, E])

            # top-8 selection via DVE max8 + match_replace
            rep = work.tile([P, NCH, E], F32, name="rep")
            for c in range(NCH):
                mx8 = work.tile([P, 8], F32, name="mx8")
                nc.vector.max(out=mx8[:], in_=rl_all[:, c, :])
                nc.vector.match_replace(out=rep[:, c, :], in_to_replace=mx8[:],
                                        in_values=rl_all[:, c, :], imm_value=-1e30)

            # softmax over E
            mxn = work.tile([P, NCH, 1], F32, name="mxn")
            nc.vector.tensor_reduce(out=mxn[:, :, 0], in_=rl_all[:], axis=AX.X,
                                    op=ALU.max, negate=True)
            ex = work.tile([P, NCH, E], F32, name="ex")
            for c in range(NCH):
                nc.scalar.activation(out=ex[:, c, :], in_=rl_all[:, c, :],
                                     func=ACTF.Exp, bias=mxn[:, c, 0:1], scale=1.0)
            sm = work.tile([P, NCH, 1], F32, name="sm")
            nc.vector.tensor_reduce(out=sm[:, :, 0], in_=ex[:], axis=AX.X,
                                    op=ALU.add)
            inv = work.tile([P, NCH, 1], F32, name="inv")
            nc.vector.reciprocal(out=inv[:], in_=sm[:])

            cmb = work.tile([P, NCH, E], F32, name="cmb")
            nc.vector.tensor_tensor(out=cmb[:], in0=rl_all[:], in1=rep[:],
                                    op=ALU.not_equal)
            nc.vector.tensor_tensor(out=cmb[:], in0=cmb[:], in1=ex[:],
                                    op=ALU.mult)
            nc.vector.tensor_tensor(out=cmb[:], in0=cmb[:], in1=bcast(inv),
                                    op=ALU.mult)
            for c in range(NCH):
                nc.sync.dma_start(out=combo[c], in_=cmb[:, c, :])
    nc.compile()
    return nc


# ---------------------------------------------------------------------------
# K2: expert kernel (expert-parallel; C tokens per expert, compile-time C)
# ---------------------------------------------------------------------------
def build_k2(C):
    CR = P * math.ceil(C / P)  # down-proj processes 128-token chunks
    CC = CR // P
    CS = slice_plan(C)    # gate/up column slices
    KH = H // P           # 16
    KF = F // P           # 8
    MF = F // P           # 8 m-chunks for gate/up
    HS = H // 512         # 4

    nc = bacc.Bacc(None, target_bir_lowering=False)
    # xgt per expert: two k-half blocks, each [128, KH/2, C] raveled
    xgt = nc.dram_tensor("xgt", [E_LOC, 2, P * (KH // 2) * C], BF16,
                         kind="ExternalInput")
    # gate+up weights packed per m-chunk: [e, m, p, {g,u}, k, f]
    wgut = nc.dram_tensor("wgut", [E_LOC, MF, P, 2, KH, P], BF16,
                          kind="ExternalInput")
    # down weights: [e, p, hs-pair, {hs}, k, f512]
    wdt = nc.dram_tensor("wdt", [E_LOC, P, 2, 2, KF, 512], BF16,
                         kind="ExternalInput")
    wv = nc.dram_tensor("wv", [E_LOC, P, CC], F32, kind="ExternalInput")
    outc = nc.dram_tensor("outc", [E_LOC, CC, HS, P, 512], BF16, kind="ExternalOutput")

    with tile.TileContext(nc) as tc:
        with tc.tile_pool(name="xg", bufs=2) as xg_pool, \
             tc.tile_pool(name="act", bufs=2) as act_pool, \
             tc.tile_pool(name="wgu", bufs=2) as wgu_pool, \
             tc.tile_pool(name="wd", bufs=2) as wd_pool, \
             tc.tile_pool(name="wvp", bufs=2) as wv_pool, \
             tc.tile_pool(name="tmp", bufs=3) as tmp_pool, \
             tc.tile_pool(name="ev", bufs=4) as ev_pool, \
             tc.tile_pool(name="psg", bufs=2, space="PSUM") as psg, \
             tc.tile_pool(name="psu", bufs=2, space="PSUM") as psu, \
             tc.tile_pool(name="psd", bufs=3, space="PSUM") as psd:
            # PE warmup: keep HAM at 8/8 while initial DMAs land
            warm = tmp_pool.tile([P, 384], BF16, name="warm")
            nc.vector.memset(warm[:], 0.0)
            for _ in range(20):
                wps = psd.tile([P, 512], F32, name="pd")
                nc.tensor.matmul(out=wps[:, :384], lhsT=warm[:, :P],
                                 rhs=warm[:, :384], start=True, stop=True)

            for e in range(E_LOC):
                xgt_sb = xg_pool.tile([P, KH, C], BF16, name="xgt_sb")
                nc.sync.dma_start(out=xgt_sb[:, :KH // 2], in_=xgt[e, 0])
                nc.gpsimd.dma_start(out=xgt_sb[:, KH // 2:], in_=xgt[e, 1])
                wv_sb = wv_pool.tile([P, CC], F32, name="wv_sb")
                nc.gpsimd.dma_start(out=wv_sb[:], in_=wv[e])
                actT = act_pool.tile([P, KF, CR], BF16, name="actT")
                if CR > C:
                    # zero the padded token tail so the down matmul reads 0s
                    nc.vector.memset(actT[:, :, C:], 0.0)

                # gate/up projections + silu*up, output actT [F, C]
                for m in range(MF):
                    wgu_sb = wgu_pool.tile([P, 2, KH, P], BF16, name="wgu_sb")
                    nc.scalar.dma_start(out=wgu_sb[:], in_=wgut[e, m])
                    for (c0, cw) in CS:
                        pg = psg.tile([P, 512], F32, name="pg")[:, :cw]
                        pu = psu.tile([P, 512], F32, name="pu")[:, :cw]
                        for k in range(KH):
                            nc.tensor.matmul(
                                out=pg[:], lhsT=wgu_sb[:, 0, k, :],
                                rhs=xgt_sb[:, k, c0:c0 + cw],
                                start=(k == 0), stop=(k == KH - 1))
                        for k in range(KH):
                            nc.tensor.matmul(
                                out=pu[:], lhsT=wgu_sb[:, 1, k, :],
                                rhs=xgt_sb[:, k, c0:c0 + cw],
                                start=(k == 0), stop=(k == KH - 1))
                        sg = tmp_pool.tile([P, 512], F32, name="sg")[:, :cw]
                        nc.scalar.activation(out=sg[:], in_=pg[:], func=ACTF.Silu,
                                             bias=0.0, scale=1.0)
                        nc.vector.tensor_tensor(
                            out=actT[:, m, c0:c0 + cw], in0=sg[:], in1=pu[:],
                            op=ALU.mult)

                # down projection, gating scale at eviction, compact out rows
                for hc in range(2):
                    wd_sb = wd_pool.tile([P, 2, KF, 512], BF16, name="wd_sb")
                    nc.gpsimd.dma_start(out=wd_sb[:], in_=wdt[e, :, hc])
                    for hi in range(2):
                        hs = hc * 2 + hi
                        for cc in range(CC):
                            pd = psd.tile([P, 512], F32, name="pd")
                            for k in range(KF):
                                nc.tensor.matmul(
                                    out=pd[:], lhsT=actT[:, k, cc * P:(cc + 1) * P],
                                    rhs=wd_sb[:, hi, k, :],
                                    start=(k == 0), stop=(k == KF - 1))
                            ev = ev_pool.tile([P, 512], BF16, name="ev")
                            nc.scalar.activation(out=ev[:], in_=pd[:], func=ACTF.Copy,
                                                 bias=0.0, scale=wv_sb[:, cc:cc + 1])
                            evq = nc.sync if (cc % 2 == 0) else nc.scalar
                            evq.dma_start(out=outc[e, cc, hs], in_=ev[:])
    nc.compile()
    return nc


# ---------------------------------------------------------------------------
# host orchestration
# ---------------------------------------------------------------------------
def _il(x, p=P):
    """[R, N] -> [p, R//p, N] with row r = k*p + part."""
    r, n = x.shape
    return np.ascontiguousarray(x.reshape(r // p, p, n).transpose(1, 0, 2))


_k2_cache = {}


def kernel(hidden_states, gumbel_u, W1, b1, W2, b2, gate_w, U, alpha, Wg, Wu, Wd):
    import time as _time

    t_start = _time.time()
    x = np.asarray(hidden_states, np.float32).reshape(T, H)

    # ---- host prep for K1 ----
    # xT interleaved: [128, H/128, T] with (p, k, t) = x[t, k*128+p]
    xT_il = np.ascontiguousarray(np.asarray(x).reshape(T, H // P, P).transpose(2, 1, 0))
    xT_hi = xT_il.astype(BF)
    xT_lo = (xT_il - xT_hi.astype(np.float32)).astype(BF)
    w1t = np.asarray(W1, np.float32).reshape(M // P, P, H // P, P).transpose(0, 3, 2, 1)
    w1t = np.ascontiguousarray(w1t).astype(BF)
    w2t = _il(np.ascontiguousarray(np.asarray(W2, np.float32).T)).astype(BF)
    gwt = _il(np.ascontiguousarray(np.asarray(gate_w, np.float32).T))    # [128, 16, 16]
    gwh = gwt.astype(BF)
    gwl = (gwt - gwh.astype(np.float32)).astype(BF)
    au = (np.float32(alpha) * np.asarray(U, np.float32)).astype(BF)
    guT = np.ascontiguousarray(np.asarray(gumbel_u, np.float32).T)       # [8, T]
    b1t = np.ascontiguousarray(np.asarray(b1, np.float32).reshape(M // P, P).T)
    b2t = np.ascontiguousarray(np.asarray(b2, np.float32).reshape(Z, 1))

    in_maps1 = []
    for c in range(N_CORES):
        sl = slice(c * TC, (c + 1) * TC)
        in_maps1.append({
            "xt": np.ascontiguousarray(xT_hi[:, :, sl]),
            "xlo": np.ascontiguousarray(xT_lo[:, :, sl]),
            "w1t": w1t, "w2t": w2t, "gwh": gwh, "gwl": gwl, "au": au,
            "gut": np.ascontiguousarray(guT[:, sl]),
            "b1t": b1t, "b2t": b2t,
        })

    t0 = _time.time()
    nc1 = _k2_cache.get("k1")
    if nc1 is None:
        nc1 = build_k1()
        _k2_cache["k1"] = nc1
    _timings["k1_build"] = _time.time() - t0

    t0 = _time.time()
    res1 = run_bass_kernel_spmd(nc1, in_maps1, list(range(N_CORES)), trace=TRACE)
    _timings["k1_run"] = _time.time() - t0
    if TRACE:
        _timings["k1_hw_ns"] = res1.exec_time_ns

    comb = np.concatenate(
        [res1.results[c]["combo"].reshape(TC, E) for c in range(N_CORES)], axis=0)

    # ---- host routing: index lists + dispatch ----
    t0 = _time.time()
    idxs, wvals, counts = [], [], []
    for e in range(E):
        ie = np.nonzero(comb[:, e] > 0)[0].astype(np.int64)
        idxs.append(ie)
        wvals.append(comb[ie, e].astype(np.float32))
        counts.append(len(ie))
    C = max(128, 64 * math.ceil(max(counts) / 64))
    CC = math.ceil(C / P)
    CR = CC * P

    idx_pad = np.zeros((E, C), np.int64)
    w_pad = np.zeros((E, CR), np.float32)
    for e in range(E):
        n = counts[e]
        idx_pad[e, :n] = idxs[e]
        w_pad[e, :n] = wvals[e]

    # weights, transposed+interleaved+blocked (built once; per-core slices are views)
    MF, HS = F // P, H // 512
    # [E, MF, 128(p), 16(k), 128(f)]: wgt[e,m,p,k,j] = Wg[e, m*128+j, k*128+p]
    WgT = np.asarray(Wg, np.float32).reshape(E, MF, P, H // P, P).transpose(0, 1, 4, 3, 2)
    WuT = np.asarray(Wu, np.float32).reshape(E, MF, P, H // P, P).transpose(0, 1, 4, 3, 2)
    # gate+up packed per m: [E, MF, P, {g,u}, KH, P]
    Wgu = np.stack([WgT.astype(BF), WuT.astype(BF)], axis=3)
    # [E, HS, 128(p), 8(k), 512(h)]: wdt[e,s,p,k,j] = Wd[e, s*512+j, k*128+p]
    WdT = np.asarray(Wd, np.float32).reshape(E, HS, 512, F // P, P).transpose(0, 1, 4, 3, 2)
    # -> [E, P, 2(hc), 2(hi), KF, 512] for single-dma-per-hs-pair loading
    WdT2 = np.ascontiguousarray(WdT.astype(BF).transpose(0, 2, 1, 3, 4)).reshape(
        E, P, 2, 2, F // P, 512)

    in_maps2 = []
    for c in range(N_CORES):
        es = [E_LOC * c + i for i in range(E_LOC)]
        xg_list = []
        for e in es:
            g = xT_hi[:, :, idx_pad[e]]                               # [128,16,C]
            xg_list.append(np.stack([
                np.ascontiguousarray(g[:, :8, :]).reshape(-1),
                np.ascontiguousarray(g[:, 8:, :]).reshape(-1)]))
        xg = np.stack(xg_list)                                        # [2,2,P*8*C]
        wvc = np.stack([np.ascontiguousarray(w_pad[e].reshape(CC, P).T)
                        for e in es])                                  # [2,128,CC]
        in_maps2.append({
            "xgt": xg,
            "wgut": Wgu[es[0]:es[-1] + 1],
            "wdt": WdT2[es[0]:es[-1] + 1],
            "wv": wvc,
        })
    _timings["dispatch"] = _time.time() - t0

    t0 = _time.time()
    nc2 = _k2_cache.get(("k2", C))
    if nc2 is None:
        nc2 = build_k2(C)
        _k2_cache[("k2", C)] = nc2
    _timings["k2_build"] = _time.time() - t0

    t0 = _time.time()
    res2 = run_bass_kernel_spmd(nc2, in_maps2, list(range(N_CORES)), trace=TRACE)
    _timings["k2_run"] = _time.time() - t0
    if TRACE:
        _timings["k2_hw_ns"] = res2.exec_time_ns

    # ---- host combine (unshard) ----
    t0 = _time.time()
    y = np.zeros((T, H), np.float32)
    for e in range(E):
        c, i = divmod(e, E_LOC)
        oc = res2.results[c]["outc"][i]          # [CC, HS, 128, 512] bf16
        oc = oc.transpose(0, 2, 1, 3).reshape(-1, H).astype(np.float32)
        n = counts[e]
        y[idxs[e]] += oc[:n]
    _timings["combine"] = _time.time() - t0
    _timings["total"] = _time.time() - t_start
    return y.reshape(B, S, H)
